# revision 27
# baseline (speedup 1.0000x reference)
"""Bezier2Image Trainium2 kernel (Bass/Tile, 8-core data parallel).

Computation per sample b:
  ctrl = x[b].reshape(160, 4, 2); pts = T @ ctrl  -> 4800 (curve, t) points
  gX[p, w] = exp(-(bX_w - X_p)^2 / ALPHA), gY likewise  (separable splat)
  out[b] = min(gX^T @ gY, 1)   (contraction over the 4800 points)

Device mapping (per core, 16 samples), v2 — ACT-walled design:
  - points in 40 chunks of 120 (4 curves x 30 samples), partition dim =
    point-within-chunk
  - nkxy[p, (c, t)] = -KS * pts: ONE fp32 matmul (Wc @ staged) into PSUM
  - ds[p, c, t, w] = KS*bX_w - KS*pts  (bf16): one broadcast tensor_tensor,
    split DVE (56 of 80 (c,t)-units) / GPSIMD (24 units) to keep both
    under the ACT wall; GPSIMD reads a DVE-copied SBUF mirror of nkxy
  - gaussians: ONE ACT pass per sample over [120, 4800]:
    Derivative_Erf(x) = (2/sqrt(pi)) * exp(-x^2); the (4/pi) factor on
    gX*gY is undone in the epilogue.  ACT busy = 16*(224+4800)/1.2 ~ 67us
    = the roofline for this kernel (only ACT can exp; 9.2M exps/core).
  - accumulation: 40 bf16 matmuls [120x60]^T @ [120x60] into one PSUM bank
  - epilogue on DVE: min(res * pi/4, 1), DMA out
"""

import numpy as np

N = 30
W = 60
LENGTH = 160
ALPHA = 2e-4
B = 128
NCORES = 8
BPC = B // NCORES  # samples per core
KS = float(1.0 / np.sqrt(ALPHA))
NCH = 40  # chunks per sample
PCH = 120  # points per chunk (4 curves x 30)
# ds work split over the 80 (chunk, coord) units: DVE computes [0, DVE_U),
# GPSIMD computes [DVE_U, 80).  56/24 balances both engines just under the
# ACT wall (measured best; all-DVE and 60/20 are worse — GPSIMD genuinely
# overlaps despite sharing one SBUF port with DVE).
HALF_U = 40
DVE_U = 56

_state = {}


def _bezier_T():
    t = np.arange(N, dtype=np.float64) / N
    t = 2.0 * t**3 - 3.0 * t**2 + 2.0 * t
    t3 = t**3
    T = np.stack(
        [t3, 3.0 * (t**2 - t3), 3.0 * (t3 - 2.0 * t**2 + t), (1.0 - t) ** 3],
        axis=1,
    )
    return T  # [N, 4] float64


def build_nc(
    loop_n=1, sim_safe=False, ablate=(), dve_u=DVE_U, split_act=False, pair_act=False
):
    ablate = frozenset(ablate)
    from contextlib import ExitStack

    import concourse.bacc as bacc
    import concourse.mybir as mybir
    import concourse.tile as tile

    fp32 = mybir.dt.float32
    bf16 = mybir.dt.bfloat16
    AF = mybir.ActivationFunctionType

    # Bacc (not plain Bass): its compile() pass splits multi-sem waits into
    # event-semaphore instructions — walrus codegen allows only one sync wait
    # per compute instruction.
    nc = bacc.Bacc()
    x_in = nc.declare_dram_parameter("x", [BPC, LENGTH, 8], fp32, isOutput=False)
    out_d = nc.declare_dram_parameter("out", [BPC, W, W], fp32, isOutput=True)

    # Constants.
    T = _bezier_T()  # [30, 4]
    q = np.arange(PCH)
    # Wc[(dl, k), q] = -KS * T[q % 30, k] if q // 30 == dl else 0.
    # One matmul Wc.T @ staged then computes -KS * pts for a whole
    # sample: nkXY[q, (c,t)] = sum_{dl,k} Wc[(dl,k), q] * x[b, 4c+dl, 2k+t].
    Wc_np = np.zeros((16, PCH), np.float32)
    for dl in range(4):
        for k in range(4):
            row = np.where(q // N == dl, -KS * T[q % N, k], 0.0)
            Wc_np[dl * 4 + k] = row.astype(np.float32)
    bxk_np = np.broadcast_to(
        (KS * np.arange(W, dtype=np.float64) / W).astype(np.float32), (128, W)
    ).copy()

    Wc_d = nc.inline_tensor(Wc_np, "Wc")
    bxk_d = nc.inline_tensor(bxk_np, "bxk")

    with ExitStack() as ctx:
        tc = ctx.enter_context(tile.TileContext(nc))
        consts = ctx.enter_context(tc.tile_pool(name="consts", bufs=1))
        small = ctx.enter_context(tc.tile_pool(name="small", bufs=4))
        big = ctx.enter_context(tc.tile_pool(name="big", bufs=4))
        psum = ctx.enter_context(tc.tile_pool(name="psum", bufs=3, space="PSUM"))
        psum_pts = ctx.enter_context(tc.tile_pool(name="psum_pts", bufs=3, space="PSUM"))
        outp = ctx.enter_context(tc.tile_pool(name="outp", bufs=6))

        # Warmups — emitted before the sample loop, so they run once per
        # invocation during the initial DMA fill and cost nothing at steady
        # state:
        #  1. A tiny activation forces the Derivative_Erf table set to DMA
        #     into ACT's table RAMs (~2.7us) under the fill instead of
        #     serializing before the first real activation.
        #  2. ~35 dummy matmuls (~1.8us, shorter than the staged-DMA
        #     latency they hide under) start the PE HAM activity window so
        #     the clock-gate reaches 8/8 sooner than the first real
        #     matmuls alone would manage.
        warm_in = consts.tile([2, 64], bf16)
        nc.vector.memset(warm_in, 0.0)
        warm_out = consts.tile([2, 64], bf16)
        if not sim_safe:
            nc.scalar.activation(warm_out, warm_in, AF.Derivative_Erf)
        warm_ps = psum_pts.tile([PCH, NCH * 2], fp32, name="warm_ps", tag="nkxy")
        for i in range(35):
            nc.tensor.matmul(
                warm_ps[:64, :64], warm_in, warm_in, start=(i == 0), stop=(i == 34)
            )

        # Consts ride the GPSIMD/ACT DMA queues so sample 0's staged DMA (on
        # the sync queue) issues immediately and the pipeline fills sooner.
        Wc = consts.tile([16, PCH], fp32)
        nc.gpsimd.dma_start(out=Wc, in_=Wc_d[:, :])
        bxk = consts.tile([128, W], fp32)
        nc.scalar.dma_start(out=bxk, in_=bxk_d[:, :])

        loop_ctx = tc.For_i(0, loop_n, 1) if loop_n > 1 else None
        if loop_ctx is not None:
            ctx.enter_context(loop_ctx)

        def emit_tail(bb, get_chunk):
            res = psum.tile([W, W], fp32, name=f"res_{bb}", tag="res")
            mm_chunks = (0, NCH - 1) if "mm" in ablate else tuple(range(NCH))
            for i, c in enumerate(mm_chunks):
                g0, g1 = get_chunk(c)
                nc.tensor.matmul(
                    res, g0, g1, start=(i == 0), stop=(i == len(mm_chunks) - 1)
                )
            res_sb = outp.tile([W, W], fp32, name=f"rs_{bb}", tag="res_sb")
            # res carries the (2/sqrt(pi))^2 factor from Derivative_Erf:
            # undo with *pi/4, then clamp.
            nc.vector.tensor_scalar(
                res_sb,
                res,
                float(np.pi / 4.0),
                1.0,
                op0=mybir.AluOpType.mult,
                op1=mybir.AluOpType.min,
            )
            nc.sync.dma_start(out=out_d[bb], in_=res_sb)

        def chunk_getter(g_s):
            gv = g_s.rearrange("q (c t) w -> q c t w", t=2)
            return lambda c: (gv[:, c, 0], gv[:, c, 1])

        _deferred_tail = None
        for b in range(BPC):
            # staged[(dl,k), c, t] = x[b, 4c+dl, 2k+t]
            staged = small.tile([16, NCH, 2], fp32)
            nc.sync.dma_start(
                out=staged,
                in_=x_in[b].rearrange("(c dl) (k t) -> (dl k) c t", dl=4, t=2),
            )
            # nkxy[q, (c, t)] = -KS * pts[l(q,c), n(q), t]
            nkxy = psum_pts.tile([PCH, NCH * 2], fp32, name=f"nkxy_{b}", tag="nkxy")
            nc.tensor.matmul(nkxy, Wc, staged.rearrange("k c t -> k (c t)"))
            if dve_u < NCH * 2:
                # SBUF mirror for GPSIMD (it cannot read PSUM).
                nkxy_sb = small.tile(
                    [PCH, NCH * 2], fp32, name=f"nkxysb_{b}", tag="nkxy_sb"
                )
                nc.vector.tensor_copy(nkxy_sb, nkxy)

            # ds/g tiles are allocated per PAIR of samples; ACT instructions
            # slice them flexibly: one big [120, 9600] instruction per middle
            # pair (amortizes the ~190ns per-instruction SBUF bubble), but
            # the first sample is split into halves (pipeline fills ~2us
            # sooner) and the last sample too (its res matmuls overlap the
            # second ACT half, shortening the drain).
            # First and last samples use four SEPARATE quarter tiles so the
            # dependencies are tile-granular: ACT starts after only 20 units
            # of DVE work (shorter fill), and the last sample's res matmuls
            # for early quarters overlap the later ACT quarters (shorter
            # drain).  Quarter boundaries 20/40/56 are even, so res-matmul
            # chunks map cleanly (c<10 / <20 / <28 / <40).
            edge = (
                b in (0, BPC - 1)
                and not sim_safe
                and "ds" not in ablate
                and "act" not in ablate
            )
            if edge:
                QR = ((0, 20), (20, 40), (40, 56), (56, 80))
                gqs = []
                for qi, (lo, hi) in enumerate(QR):
                    dsq = big.tile(
                        [PCH, hi - lo, W], bf16, name=f"dsq{qi}_{b}", tag=f"dsq{qi}"
                    )
                    if qi < 3:
                        nc.vector.tensor_add(
                            dsq,
                            bxk[:PCH].unsqueeze(1).broadcast_to([PCH, hi - lo, W]),
                            nkxy[:, lo:hi].unsqueeze(2).broadcast_to(
                                [PCH, hi - lo, W]
                            ),
                        )
                    else:
                        nc.gpsimd.tensor_add(
                            dsq,
                            bxk[:PCH].unsqueeze(1).broadcast_to([PCH, hi - lo, W]),
                            nkxy_sb[:, lo:hi].unsqueeze(2).broadcast_to(
                                [PCH, hi - lo, W]
                            ),
                        )
                    gq = big.tile(
                        [PCH, hi - lo, W], bf16, name=f"gq{qi}_{b}", tag=f"gq{qi}"
                    )
                    nc.scalar.activation(gq, dsq, AF.Derivative_Erf)
                    gqs.append(gq)

                def get_chunk_edge(c, gqs=gqs):
                    qi = 0 if c < 10 else (1 if c < 20 else (2 if c < 28 else 3))
                    lo = QR[qi][0]
                    gv = gqs[qi].rearrange("q (c t) w -> q c t w", t=2)
                    return gv[:, c - lo // 2, 0], gv[:, c - lo // 2, 1]

                emit_tail(b, get_chunk_edge)
                continue

            if pair_act:
                if b % 2 == 0:
                    ds_pair = big.tile(
                        [PCH, 2, NCH * 2, W], bf16, name=f"ds_{b}", tag="ds"
                    )
                    g_pair = big.tile(
                        [PCH, 2, NCH * 2, W], bf16, name=f"g_{b}", tag="g"
                    )
                    _state_pair = (ds_pair, g_pair)
                else:
                    ds_pair, g_pair = _state_pair
                ds = ds_pair[:, b % 2]
                g = g_pair[:, b % 2]
            else:
                ds = big.tile([PCH, NCH * 2, W], bf16, name=f"ds_{b}", tag="ds")
                g = big.tile([PCH, NCH * 2, W], bf16, name=f"g_{b}", tag="g")

            if "ds" not in ablate:
                dve_ranges = (
                    ((0, HALF_U), (HALF_U, dve_u)) if split_act else ((0, dve_u),)
                )
                for lo, hi in dve_ranges:
                    nc.vector.tensor_add(
                        ds[:, lo:hi],
                        bxk[:PCH].unsqueeze(1).broadcast_to([PCH, hi - lo, W]),
                        nkxy[:, lo:hi].unsqueeze(2).broadcast_to([PCH, hi - lo, W]),
                    )
                if dve_u < NCH * 2:
                    nc.gpsimd.tensor_add(
                        ds[:, dve_u:],
                        bxk[:PCH].unsqueeze(1).broadcast_to([PCH, NCH * 2 - dve_u, W]),
                        nkxy_sb[:, dve_u:].unsqueeze(2).broadcast_to(
                            [PCH, NCH * 2 - dve_u, W]
                        ),
                    )

            # ACT pass(es): g = (2/sqrt(pi)) * exp(-ds^2).  For merged middle
            # pairs the single pair-wide instruction is emitted on the ODD
            # sample, and the even sample's tail is deferred past it (Tile
            # dependency tracking is emission-ordered).
            act_emitted = True
            if sim_safe:
                # CoreSim lacks Derivative_Erf: equivalent two-op path.
                d2 = big.tile([PCH, NCH * 2, W], bf16, name=f"d2_{b}", tag="d2")
                nc.vector.tensor_mul(d2, ds, ds)
                nc.scalar.activation(g, d2, AF.Exp, scale=-1.0)
                nc.vector.tensor_scalar_mul(g, g, float(2.0 / np.sqrt(np.pi)))
            elif "act" in ablate:
                g = ds
            elif b == 0 or b == BPC - 1 or split_act:
                nc.scalar.activation(g[:, :HALF_U], ds[:, :HALF_U], AF.Derivative_Erf)
                nc.scalar.activation(g[:, HALF_U:], ds[:, HALF_U:], AF.Derivative_Erf)
            elif not pair_act or b == 1 or b == BPC - 2:
                # full-sample instruction
                nc.scalar.activation(g, ds, AF.Derivative_Erf)
            elif b % 2 == 1:
                # merged: one instruction covers this sample and the previous
                nc.scalar.activation(g_pair, ds_pair, AF.Derivative_Erf)
            else:
                act_emitted = False  # covered by the pair ACT next iteration

            if act_emitted:
                if _deferred_tail is not None:
                    emit_tail(_deferred_tail[0], chunk_getter(_deferred_tail[1]))
                    _deferred_tail = None
                emit_tail(b, chunk_getter(g))
            else:
                _deferred_tail = (b, g)

    nc.compile()
    return nc


def kernel(x):
    import os

    x = np.ascontiguousarray(x, dtype=np.float32)
    assert x.shape == (B, LENGTH, 8), x.shape
    if "nc" not in _state:
        _state["nc"] = build_nc()
    from concourse.bass_utils import run_bass_kernel_spmd

    in_maps = [{"x": x[i * BPC : (i + 1) * BPC]} for i in range(NCORES)]
    trace = bool(os.environ.get("BEZIER_TRACE"))
    res = run_bass_kernel_spmd(
        _state["nc"], in_maps, core_ids=list(range(NCORES)), trace=trace
    )
    _state["last_results"] = res
    return np.concatenate([r["out"] for r in res.results], axis=0)


# revision 28
# speedup vs baseline: 1.0405x; 1.0405x over previous
"""Bezier2Image Trainium2 kernel (Bass/Tile, 8-core data parallel).

Computation per sample b:
  ctrl = x[b].reshape(160, 4, 2); pts = T @ ctrl  -> 4800 (curve, t) points
  gX[p, w] = exp(-(bX_w - X_p)^2 / ALPHA), gY likewise  (separable splat)
  out[b] = min(gX^T @ gY, 1)   (contraction over the 4800 points)

Device mapping (per core, 16 samples), v2 — ACT-walled design:
  - points in 40 chunks of 120 (4 curves x 30 samples), partition dim =
    point-within-chunk
  - nkxy[p, (c, t)] = -KS * pts: ONE fp32 matmul (Wc @ staged) into PSUM
  - ds[p, c, t, w] = KS*bX_w - KS*pts  (bf16): one broadcast tensor_tensor,
    split DVE (56 of 80 (c,t)-units) / GPSIMD (24 units) to keep both
    under the ACT wall; GPSIMD reads a DVE-copied SBUF mirror of nkxy
  - gaussians: ONE ACT pass per sample over [120, 4800]:
    Derivative_Erf(x) = (2/sqrt(pi)) * exp(-x^2); the (4/pi) factor on
    gX*gY is undone in the epilogue.  ACT busy = 16*(224+4800)/1.2 ~ 67us
    = the roofline for this kernel (only ACT can exp; 9.2M exps/core).
  - accumulation: 40 bf16 matmuls [120x60]^T @ [120x60] into one PSUM bank
  - epilogue on DVE: min(res * pi/4, 1), DMA out
"""

import numpy as np

N = 30
W = 60
LENGTH = 160
ALPHA = 2e-4
B = 128
NCORES = 8
BPC = B // NCORES  # samples per core
KS = float(1.0 / np.sqrt(ALPHA))
NCH = 40  # chunks per sample
PCH = 120  # points per chunk (4 curves x 30)
# ds work split over the 80 (chunk, coord) units: DVE computes [0, DVE_U),
# GPSIMD computes [DVE_U, 80).  56/24 balances both engines just under the
# ACT wall (measured best; all-DVE and 60/20 are worse — GPSIMD genuinely
# overlaps despite sharing one SBUF port with DVE).
HALF_U = 40
DVE_U = 56

_state = {}


def _bezier_T():
    t = np.arange(N, dtype=np.float64) / N
    t = 2.0 * t**3 - 3.0 * t**2 + 2.0 * t
    t3 = t**3
    T = np.stack(
        [t3, 3.0 * (t**2 - t3), 3.0 * (t3 - 2.0 * t**2 + t), (1.0 - t) ** 3],
        axis=1,
    )
    return T  # [N, 4] float64


def build_nc(
    loop_n=1, sim_safe=False, ablate=(), dve_u=DVE_U, split_act=False, pair_act=False
):
    ablate = frozenset(ablate)
    from contextlib import ExitStack

    import concourse.bacc as bacc
    import concourse.mybir as mybir
    import concourse.tile as tile

    fp32 = mybir.dt.float32
    bf16 = mybir.dt.bfloat16
    AF = mybir.ActivationFunctionType

    # Bacc (not plain Bass): its compile() pass splits multi-sem waits into
    # event-semaphore instructions — walrus codegen allows only one sync wait
    # per compute instruction.
    nc = bacc.Bacc()
    x_in = nc.declare_dram_parameter("x", [BPC, LENGTH, 8], fp32, isOutput=False)
    out_d = nc.declare_dram_parameter("out", [BPC, W, W], fp32, isOutput=True)

    # Constants.
    T = _bezier_T()  # [30, 4]
    q = np.arange(PCH)
    # Wc[(dl, k), q] = -KS * T[q % 30, k] if q // 30 == dl else 0.
    # One matmul Wc.T @ staged then computes -KS * pts for a whole
    # sample: nkXY[q, (c,t)] = sum_{dl,k} Wc[(dl,k), q] * x[b, 4c+dl, 2k+t].
    Wc_np = np.zeros((16, PCH), np.float32)
    for dl in range(4):
        for k in range(4):
            row = np.where(q // N == dl, -KS * T[q % N, k], 0.0)
            Wc_np[dl * 4 + k] = row.astype(np.float32)
    bxk_np = np.broadcast_to(
        (KS * np.arange(W, dtype=np.float64) / W).astype(np.float32), (128, W)
    ).copy()

    Wc_d = nc.inline_tensor(Wc_np, "Wc")
    bxk_d = nc.inline_tensor(bxk_np, "bxk")

    with ExitStack() as ctx:
        tc = ctx.enter_context(tile.TileContext(nc))
        consts = ctx.enter_context(tc.tile_pool(name="consts", bufs=1))
        small = ctx.enter_context(tc.tile_pool(name="small", bufs=4))
        big = ctx.enter_context(tc.tile_pool(name="big", bufs=4))
        psum = ctx.enter_context(tc.tile_pool(name="psum", bufs=3, space="PSUM"))
        psum_pts = ctx.enter_context(tc.tile_pool(name="psum_pts", bufs=3, space="PSUM"))
        outp = ctx.enter_context(tc.tile_pool(name="outp", bufs=6))

        # Warmups — emitted before the sample loop, so they run once per
        # invocation during the initial DMA fill and cost nothing at steady
        # state:
        #  1. A tiny activation forces the Derivative_Erf table set to DMA
        #     into ACT's table RAMs (~2.7us) under the fill instead of
        #     serializing before the first real activation.
        #  2. ~35 dummy matmuls (~1.8us, shorter than the staged-DMA
        #     latency they hide under) start the PE HAM activity window so
        #     the clock-gate reaches 8/8 sooner than the first real
        #     matmuls alone would manage.
        warm_in = consts.tile([2, 64], bf16)
        nc.vector.memset(warm_in, 0.0)
        warm_out = consts.tile([2, 64], bf16)
        if not sim_safe:
            nc.scalar.activation(warm_out, warm_in, AF.Derivative_Erf)
        warm_ps = psum_pts.tile([PCH, NCH * 2], fp32, name="warm_ps", tag="nkxy")
        for i in range(35):
            nc.tensor.matmul(
                warm_ps[:64, :64], warm_in, warm_in, start=(i == 0), stop=(i == 34)
            )

        # Consts ride the GPSIMD/ACT DMA queues so sample 0's staged DMA (on
        # the sync queue) issues immediately and the pipeline fills sooner.
        Wc = consts.tile([16, PCH], fp32)
        nc.gpsimd.dma_start(out=Wc, in_=Wc_d[:, :])
        bxk = consts.tile([128, W], fp32)
        nc.scalar.dma_start(out=bxk, in_=bxk_d[:, :])

        loop_ctx = tc.For_i(0, loop_n, 1) if loop_n > 1 else None
        if loop_ctx is not None:
            ctx.enter_context(loop_ctx)

        def emit_tail(bb, get_chunk):
            res = psum.tile([W, W], fp32, name=f"res_{bb}", tag="res")
            mm_chunks = (0, NCH - 1) if "mm" in ablate else tuple(range(NCH))
            for i, c in enumerate(mm_chunks):
                g0, g1 = get_chunk(c)
                nc.tensor.matmul(
                    res, g0, g1, start=(i == 0), stop=(i == len(mm_chunks) - 1)
                )
            res_sb = outp.tile([W, W], fp32, name=f"rs_{bb}", tag="res_sb")
            # res carries the (2/sqrt(pi))^2 factor from Derivative_Erf:
            # undo with *pi/4, then clamp.
            nc.vector.tensor_scalar(
                res_sb,
                res,
                float(np.pi / 4.0),
                1.0,
                op0=mybir.AluOpType.mult,
                op1=mybir.AluOpType.min,
            )
            nc.sync.dma_start(out=out_d[bb], in_=res_sb)

        def chunk_getter(g_s):
            gv = g_s.rearrange("q (c t) w -> q c t w", t=2)
            return lambda c: (gv[:, c, 0], gv[:, c, 1])

        _deferred_tail = None
        for b in range(BPC):
            # staged[(dl,k), c, t] = x[b, 4c+dl, 2k+t]
            staged = small.tile([16, NCH, 2], fp32)
            nc.sync.dma_start(
                out=staged,
                in_=x_in[b].rearrange("(c dl) (k t) -> (dl k) c t", dl=4, t=2),
            )
            # nkxy[q, (c, t)] = -KS * pts[l(q,c), n(q), t]
            nkxy = psum_pts.tile([PCH, NCH * 2], fp32, name=f"nkxy_{b}", tag="nkxy")
            nc.tensor.matmul(nkxy, Wc, staged.rearrange("k c t -> k (c t)"))
            if dve_u < NCH * 2:
                # SBUF mirror for GPSIMD (it cannot read PSUM).
                nkxy_sb = small.tile(
                    [PCH, NCH * 2], fp32, name=f"nkxysb_{b}", tag="nkxy_sb"
                )
                nc.vector.tensor_copy(nkxy_sb, nkxy)

            # ds/g tiles are allocated per PAIR of samples; ACT instructions
            # slice them flexibly: one big [120, 9600] instruction per middle
            # pair (amortizes the ~190ns per-instruction SBUF bubble), but
            # the first sample is split into halves (pipeline fills ~2us
            # sooner) and the last sample too (its res matmuls overlap the
            # second ACT half, shortening the drain).
            # First and last samples use four SEPARATE quarter tiles so the
            # dependencies are tile-granular: ACT starts after only 20 units
            # of DVE work (shorter fill), and the last sample's res matmuls
            # for early quarters overlap the later ACT quarters (shorter
            # drain).  Quarter boundaries 20/40/56 are even, so res-matmul
            # chunks map cleanly (c<10 / <20 / <28 / <40).
            edge = (
                b in (0, BPC - 1)
                and not sim_safe
                and "ds" not in ablate
                and "act" not in ablate
            )
            if edge:
                # Sample 0: tiny FIRST quarter (ACT starts ~0.7us sooner).
                # Last sample: tiny FINAL quarter (the closing ACT and its
                # res matmuls shrink, so the drain to the output DMA is
                # ~1us shorter).  GPSIMD owns ranges starting at >= DVE_U.
                if b == 0:
                    QR = ((0, 10), (10, 40), (40, 56), (56, 80))
                else:
                    QR = ((0, 20), (20, 40), (40, 56), (56, 72), (72, 80))
                gqs = []
                for qi, (lo, hi) in enumerate(QR):
                    dsq = big.tile(
                        [PCH, hi - lo, W], bf16, name=f"dsq{qi}_{b}", tag=f"dsq{qi}"
                    )
                    if lo < DVE_U:
                        nc.vector.tensor_add(
                            dsq,
                            bxk[:PCH].unsqueeze(1).broadcast_to([PCH, hi - lo, W]),
                            nkxy[:, lo:hi].unsqueeze(2).broadcast_to(
                                [PCH, hi - lo, W]
                            ),
                        )
                    else:
                        nc.gpsimd.tensor_add(
                            dsq,
                            bxk[:PCH].unsqueeze(1).broadcast_to([PCH, hi - lo, W]),
                            nkxy_sb[:, lo:hi].unsqueeze(2).broadcast_to(
                                [PCH, hi - lo, W]
                            ),
                        )
                    gq = big.tile(
                        [PCH, hi - lo, W], bf16, name=f"gq{qi}_{b}", tag=f"gq{qi}"
                    )
                    nc.scalar.activation(gq, dsq, AF.Derivative_Erf)
                    gqs.append(gq)

                def get_chunk_edge(c, gqs=gqs, QR=QR):
                    for qi, (lo, hi) in enumerate(QR):
                        if 2 * c < hi:
                            break
                    gv = gqs[qi].rearrange("q (c t) w -> q c t w", t=2)
                    lo2 = QR[qi][0] // 2
                    return gv[:, c - lo2, 0], gv[:, c - lo2, 1]

                emit_tail(b, get_chunk_edge)
                continue

            if pair_act:
                if b % 2 == 0:
                    ds_pair = big.tile(
                        [PCH, 2, NCH * 2, W], bf16, name=f"ds_{b}", tag="ds"
                    )
                    g_pair = big.tile(
                        [PCH, 2, NCH * 2, W], bf16, name=f"g_{b}", tag="g"
                    )
                    _state_pair = (ds_pair, g_pair)
                else:
                    ds_pair, g_pair = _state_pair
                ds = ds_pair[:, b % 2]
                g = g_pair[:, b % 2]
            else:
                ds = big.tile([PCH, NCH * 2, W], bf16, name=f"ds_{b}", tag="ds")
                g = big.tile([PCH, NCH * 2, W], bf16, name=f"g_{b}", tag="g")

            if "ds" not in ablate:
                dve_ranges = (
                    ((0, HALF_U), (HALF_U, dve_u)) if split_act else ((0, dve_u),)
                )
                for lo, hi in dve_ranges:
                    nc.vector.tensor_add(
                        ds[:, lo:hi],
                        bxk[:PCH].unsqueeze(1).broadcast_to([PCH, hi - lo, W]),
                        nkxy[:, lo:hi].unsqueeze(2).broadcast_to([PCH, hi - lo, W]),
                    )
                if dve_u < NCH * 2:
                    nc.gpsimd.tensor_add(
                        ds[:, dve_u:],
                        bxk[:PCH].unsqueeze(1).broadcast_to([PCH, NCH * 2 - dve_u, W]),
                        nkxy_sb[:, dve_u:].unsqueeze(2).broadcast_to(
                            [PCH, NCH * 2 - dve_u, W]
                        ),
                    )

            # ACT pass(es): g = (2/sqrt(pi)) * exp(-ds^2).  For merged middle
            # pairs the single pair-wide instruction is emitted on the ODD
            # sample, and the even sample's tail is deferred past it (Tile
            # dependency tracking is emission-ordered).
            act_emitted = True
            if sim_safe:
                # CoreSim lacks Derivative_Erf: equivalent two-op path.
                d2 = big.tile([PCH, NCH * 2, W], bf16, name=f"d2_{b}", tag="d2")
                nc.vector.tensor_mul(d2, ds, ds)
                nc.scalar.activation(g, d2, AF.Exp, scale=-1.0)
                nc.vector.tensor_scalar_mul(g, g, float(2.0 / np.sqrt(np.pi)))
            elif "act" in ablate:
                g = ds
            elif b == 0 or b == BPC - 1 or split_act:
                nc.scalar.activation(g[:, :HALF_U], ds[:, :HALF_U], AF.Derivative_Erf)
                nc.scalar.activation(g[:, HALF_U:], ds[:, HALF_U:], AF.Derivative_Erf)
            elif not pair_act or b == 1 or b == BPC - 2:
                # full-sample instruction
                nc.scalar.activation(g, ds, AF.Derivative_Erf)
            elif b % 2 == 1:
                # merged: one instruction covers this sample and the previous
                nc.scalar.activation(g_pair, ds_pair, AF.Derivative_Erf)
            else:
                act_emitted = False  # covered by the pair ACT next iteration

            if act_emitted:
                if _deferred_tail is not None:
                    emit_tail(_deferred_tail[0], chunk_getter(_deferred_tail[1]))
                    _deferred_tail = None
                emit_tail(b, chunk_getter(g))
            else:
                _deferred_tail = (b, g)

    nc.compile()
    return nc


def kernel(x):
    import os

    x = np.ascontiguousarray(x, dtype=np.float32)
    assert x.shape == (B, LENGTH, 8), x.shape
    if "nc" not in _state:
        _state["nc"] = build_nc()
    from concourse.bass_utils import run_bass_kernel_spmd

    in_maps = [{"x": x[i * BPC : (i + 1) * BPC]} for i in range(NCORES)]
    trace = bool(os.environ.get("BEZIER_TRACE"))
    res = run_bass_kernel_spmd(
        _state["nc"], in_maps, core_ids=list(range(NCORES)), trace=trace
    )
    _state["last_results"] = res
    return np.concatenate([r["out"] for r in res.results], axis=0)
